# revision 40
# baseline (speedup 1.0000x reference)
"""Trainium2 Bass kernel for nn_Attn_9637906612873.

Reference computation (per batch b of 32):
    Qm     = Q @ Wq.T + bq            # [TQ, DK]
    S      = Qm @ K.T                  # [TQ, L]
    P      = softmax(S, axis=-1)       # [TQ, L]  (returned as attn_weights)
    A      = P @ V                     # [TQ, DV]
    out    = concat([Q, A]) @ Wc.T + bc  # [TQ, DOUT]

Strategy: data-parallel over batch across 8 NeuronCores (4 batches/core),
weights replicated. On each core, per batch:
  phase A: Qm^T = Wq^T-chunks.T @ Q^T   (float32r, PSUM accum over q)
  phase B: S-rows per 128-row t-tile via Qm^T-chunks.T @ K^T (float32r),
           softmax on PSUM rows (DVE max, ACT exp+rowsum, DVE normalize),
           store P (f32) to attn output and P (bf16) to a DRAM scratch
  phase C: A^T = V-chunks.T @ P^T  (bf16; P^T streamed back from the
           scratch with DMA-transpose)
  phase D: out = [Q;A]^T-chunks.T @ Wc^T + bc  (bf16)

float32r = fp32 with the mantissa rounded to 11 stored bits; the PE runs it
at ~65 TF/s (vs 19 for fp32) with exact f32 accumulation, keeping softmax
logits accurate to ~4e-3 absolute. Host pre-rounds the QK-path operands so
DMA can feed float32r tiles directly.
"""
import sys

sys.path.insert(0, "/opt/trn_rl_repo")

from contextlib import ExitStack

import numpy as np
import ml_dtypes

import concourse.bass as bass
import concourse.tile as tile
from concourse import bacc, mybir
from concourse import bass_utils
from concourse.masks import make_identity

P = 128
B, TQ, L = 32, 1024, 2048
DQ, DK, DV, DOUT = 1024, 1024, 1024, 1024
NCORES = 8
BLOC = B // NCORES

f32 = mybir.dt.float32
f32r = mybir.dt.float32r
bf16 = mybir.dt.bfloat16

KC = DQ // P     # q chunks (mm1 contraction)
KT = DK // P     # k tiles
TT = TQ // P     # t tiles
LB = L // 512    # l blocks of 512
LC = L // P      # l chunks of 128
VT = DV // P     # v tiles
CC = (DQ + DV) // P  # combined chunks
OC = DOUT // 512  # out column blocks

_NC_CACHE = {}
_LAST_IN_MAPS = None


def _load_qt(nc, g, b):
    qt_sb = g["qt_pool"].tile([P, KC, TQ], f32r, name="qt_sb")
    src = g["QT"][b].rearrange("(qc p) t -> p qc t", p=P)
    for h in range(2):
        nc.gpsimd.dma_start(out=qt_sb[:, h * 4 : (h + 1) * 4, :], in_=src[:, h * 4 : (h + 1) * 4, :])
    return qt_sb


def _load_kt_first(nc, g, b):
    kt_sb = g["kt_pool"].tile([P, KC, L], f32r, name="kt_sb")
    src = g["KTd"][b].rearrange("(kc p) l -> p kc l", p=P)
    for h in range(2):
        nc.gpsimd.dma_start(out=kt_sb[:, h * 2 : (h + 1) * 2, :], in_=src[:, h * 2 : (h + 1) * 2, :])
    return kt_sb


def _load_kt_rest(nc, g, b, kt_sb):
    src = g["KTd"][b].rearrange("(kc p) l -> p kc l", p=P)
    for h in range(2, 4):
        nc.gpsimd.dma_start(out=kt_sb[:, h * 2 : (h + 1) * 2, :], in_=src[:, h * 2 : (h + 1) * 2, :])


def _prefetch_wq(nc, g):
    """First k-tile's Wq chunks, loaded ahead so phase A starts immediately."""
    tiles = []
    wq_src = g["WqT"][:, 0:P].rearrange("(qc p) k -> p qc k", p=P)
    for qh in range(2):
        w = g["wq_pool"].tile([P, KC // 2, P], f32r, name="wq_sb")
        nc.scalar.dma_start(out=w, in_=wq_src[:, qh * 4 : (qh + 1) * 4, :])
        tiles.append(w)
    return tiles


def _phase_a(nc, g, b, qt_sb, wq0):
    """QmT[k, t] = sum_q WqT[q,k] * QT[q,t] + bq  (float32r)."""
    qmt_sb = g["qmt_pool"].tile([P, KT, TQ], f32r, name="qmt_sb")
    for kt in range(KT):
        wq_src = g["WqT"][:, kt * P : (kt + 1) * P].rearrange("(qc p) k -> p qc k", p=P)
        pa = g["ps"].tile([P, 2, 512], f32, tag="ps", name="pa")
        for qh in range(2):
            if kt == 0 and wq0 is not None:
                wq_sb = wq0[qh]
            else:
                wq_sb = g["wq_pool"].tile([P, KC // 2, P], f32r, name="wq_sb")
                nc.scalar.dma_start(
                    out=wq_sb, in_=wq_src[:, qh * 4 : (qh + 1) * 4, :]
                )
            for q4 in range(KC // 2):
                qc = qh * 4 + q4
                for th in range(2):
                    nc.tensor.matmul(
                        pa[:, th, :],
                        wq_sb[:, q4, :],
                        qt_sb[:, qc, th * 512 : (th + 1) * 512],
                        start=(qc == 0),
                        stop=(qc == KC - 1),
                    )
        nc.vector.tensor_scalar_add(
            qmt_sb[:, kt, :],
            pa.rearrange("p a b -> p (a b)"),
            g["bq_sb"][:, kt : kt + 1],
        )
    g["qmt_sb"] = qmt_sb


def _transpose_pb(nc, g, pb_sb, col, pt_sb):
    """16 PE transposes of one P t-tile (bf16) into PT[l, t] columns."""
    tp = g["ps"].tile([P, LC, P], bf16, tag="ps", name="tp")
    for lc in range(LC):
        nc.tensor.transpose(
            tp[:, lc, :], pb_sb[:, lc * P : (lc + 1) * P], g["ident"][:]
        )
    nc.scalar.activation(
        pt_sb[:, :, col * P : (col + 1) * P],
        tp[:, :, :],
        mybir.ActivationFunctionType.Copy,
    )


def _phase_b(nc, g, b, th, kt_sb):
    """scores (float32r) + softmax per t-tile; PE-transpose P into PT (bf16)."""
    pt_sb = g["pt_pool"].tile([P, LC, 512], bf16, name="pt_sb")
    pending = None
    for tt in range(th * 4, th * 4 + 4):
        pb_ps = g["ps"].tile([P, LB, 512], f32, tag="ps", name="pb_ps")
        for kc in range(KC):
            lhsT = g["qmt_sb"][:, kc, tt * P : (tt + 1) * P]
            for lb in range(LB):
                nc.tensor.matmul(
                    pb_ps[:, lb, :],
                    lhsT,
                    kt_sb[:, kc, lb * 512 : (lb + 1) * 512],
                    start=(kc == 0),
                    stop=(kc == KC - 1),
                )
        if pending is not None:
            _transpose_pb(nc, g, pending[0], pending[1], pt_sb)
        flat = pb_ps.rearrange("p a b -> p (a b)")
        neg_m = g["small"].tile([P, 1], f32, name="neg_m")
        nc.vector.reduce_max(neg_m, flat, axis=mybir.AxisListType.X, negate=True)
        e_sb = g["e_pool"].tile([P, L], f32, name="e_sb")
        s_sum = g["small"].tile([P, 1], f32, name="s_sum")
        nc.scalar.activation(
            e_sb[:],
            flat,
            mybir.ActivationFunctionType.Exp,
            bias=neg_m[:],
            accum_out=s_sum[:],
        )
        r_sb = g["small"].tile([P, 1], f32, name="r_sb")
        nc.vector.reciprocal(r_sb, s_sum)
        # critical path to phase C: normalized bf16 copy straight from ACT
        pb_sb = g["pb_pool"].tile([P, L], bf16, name="pb_sb")
        nc.scalar.activation(
            pb_sb[:], e_sb[:], mybir.ActivationFunctionType.Copy, scale=r_sb[:]
        )
        pending = (pb_sb, tt - th * 4)
        # attnW output (f32), off the critical path
        nc.vector.tensor_scalar_mul(e_sb[:], e_sb[:], r_sb[:])
        nc.sync.dma_start(out=g["attnW"][b, tt * P : (tt + 1) * P, :], in_=e_sb)
    _transpose_pb(nc, g, pending[0], pending[1], pt_sb)
    return pt_sb


def _phase_c(nc, g, b, th, pt_sb):
    """attnT[v, t-half] = sum_l V[l,v] * P[t,l]  (bf16, PT resident in SBUF)."""
    attnt_sb = g["attnt_pool"].tile([P, VT, 512], bf16, name="attnt_sb")
    pc0 = g["ps"].tile([P, 4, 512], f32, tag="ps", name="pc0")
    pc1 = g["ps"].tile([P, 4, 512], f32, tag="ps", name="pc1")
    pcs = [pc0, pc1]
    for lc in range(LC):
        v_sb = g["v_pool"].tile([P, DV], bf16, name="v_sb")
        nc.gpsimd.dma_start(
            out=v_sb, in_=g["Vbf"][b, lc * P : (lc + 1) * P, :]
        )
        for vt in range(VT):
            nc.tensor.matmul(
                pcs[vt // 4][:, vt % 4, :],
                v_sb[:, vt * P : (vt + 1) * P],
                pt_sb[:, lc, :],
                start=(lc == 0),
                stop=(lc == LC - 1),
            )
    for vt in range(VT):
        nc.scalar.activation(
            attnt_sb[:, vt, :],
            pcs[vt // 4][:, vt % 4, :],
            mybir.ActivationFunctionType.Copy,
        )
    return attnt_sb


def _phase_d(nc, g, b, th, attnt_sb):
    """out[t-half, o] = sum_c comb[c,t] * WcT[c,o] + bc  (bf16)."""
    for tq in [th]:
        pd0 = g["ps"].tile([P, 4, 512], f32, tag="ps", name="pd0")
        pd1 = g["ps"].tile([P, 4, 512], f32, tag="ps", name="pd1")
        pds = [pd0, pd1]
        for cb in range(CC // 2):
            wc_sb = g["wc_pool"].tile([P, 2, DOUT], bf16, name="wc_sb")
            nc.scalar.dma_start(
                out=wc_sb,
                in_=g["WcT"][cb * 2 * P : (cb + 1) * 2 * P, :].rearrange(
                    "(c p) o -> p c o", p=P
                ),
            )
            qq = None
            if cb * 2 < KC:
                qq = g["qtbf_pool"].tile([P, 2, 512], bf16, name="qq")
                nc.scalar.dma_start(
                    out=qq,
                    in_=g["QTbf"][
                        b, cb * 2 * P : (cb + 1) * 2 * P, tq * 512 : (tq + 1) * 512
                    ].rearrange("(c p) t -> p c t", p=P),
                )
            for cc in range(2):
                c = cb * 2 + cc
                for i in range(4):
                    tt = tq * 4 + i
                    if c < KC:
                        lhsT = qq[:, cc, i * P : (i + 1) * P]
                    else:
                        lhsT = attnt_sb[:, c - KC, i * P : (i + 1) * P]
                    for oc in range(OC):
                        j = i * OC + oc
                        nc.tensor.matmul(
                            pds[j // 4][:, j % 4, :],
                            lhsT,
                            wc_sb[:, cc, oc * 512 : (oc + 1) * 512],
                            start=(c == 0),
                            stop=(c == CC - 1),
                        )
        for i in range(4):
            tt = tq * 4 + i
            for oc in range(OC):
                j = i * OC + oc
                o_sb = g["out_pool"].tile([P, 512], f32, name="o_sb")
                nc.vector.tensor_add(
                    o_sb[:],
                    pds[j // 4][:, j % 4, :],
                    g["bc_sb"][:, oc * 512 : (oc + 1) * 512],
                )
                nc.sync.dma_start(
                    out=g["out"][b, tt * P : (tt + 1) * P, oc * 512 : (oc + 1) * 512],
                    in_=o_sb,
                )


def _build():
    nc = bacc.Bacc(
        "TRN2",
        target_bir_lowering=False,
        debug=False,
        enable_asserts=True,
        num_devices=NCORES,
    )
    g = {}
    g["QT"] = nc.dram_tensor("QT", [BLOC, DQ, TQ], f32r, kind="ExternalInput").ap()
    g["KTd"] = nc.dram_tensor("KTd", [BLOC, DK, L], f32r, kind="ExternalInput").ap()
    g["Vbf"] = nc.dram_tensor("Vbf", [BLOC, L, DV], bf16, kind="ExternalInput").ap()
    g["QTbf"] = nc.dram_tensor("QTbf", [BLOC, DQ, TQ], bf16, kind="ExternalInput").ap()
    g["WqT"] = nc.dram_tensor("WqT", [DQ, DK], f32r, kind="ExternalInput").ap()
    g["bq"] = nc.dram_tensor("bq", [DK], f32, kind="ExternalInput").ap()
    g["WcT"] = nc.dram_tensor("WcT", [DQ + DV, DOUT], bf16, kind="ExternalInput").ap()
    g["bc"] = nc.dram_tensor("bc", [1, DOUT], f32, kind="ExternalInput").ap()
    g["attnW"] = nc.dram_tensor(
        "attnW", [BLOC, TQ, L], f32, kind="ExternalOutput"
    ).ap()
    g["out"] = nc.dram_tensor(
        "out", [BLOC, TQ, DOUT], f32, kind="ExternalOutput"
    ).ap()

    with tile.TileContext(nc) as tc, ExitStack() as ctx:
        for name, bufs, space in [
            ("const", 1, "SBUF"),
            ("qt_pool", 1, "SBUF"),
            ("kt_pool", 1, "SBUF"),
            ("qmt_pool", 1, "SBUF"),
            ("attnt_pool", 1, "SBUF"),
            ("wq_pool", 2, "SBUF"),
            ("e_pool", 2, "SBUF"),
            ("pb_pool", 2, "SBUF"),
            ("pt_pool", 1, "SBUF"),
            ("v_pool", 3, "SBUF"),
            ("qtbf_pool", 2, "SBUF"),
            ("wc_pool", 2, "SBUF"),
            ("out_pool", 1, "SBUF"),
            ("small", 4, "SBUF"),
            ("ps", 2, "PSUM"),
            ("dram", 2, "DRAM"),
        ]:
            g[name] = ctx.enter_context(tc.tile_pool(name=name, bufs=bufs, space=space))

        bq_sb = g["const"].tile([P, KT], f32)
        nc.sync.dma_start(out=bq_sb, in_=g["bq"].rearrange("(kt p) -> p kt", p=P))
        bc_sb = g["const"].tile([P, DOUT], f32)
        nc.sync.dma_start(out=bc_sb, in_=g["bc"].to_broadcast([P, DOUT]))
        ident = g["const"].tile([P, P], bf16)
        make_identity(nc, ident)
        g["bq_sb"] = bq_sb
        g["bc_sb"] = bc_sb
        g["ident"] = ident

        qt_cur = _load_qt(nc, g, 0)
        kt_cur = _load_kt_first(nc, g, 0)
        _load_kt_rest(nc, g, 0, kt_cur)
        wq0 = None
        for b in range(BLOC):
            _phase_a(nc, g, b, qt_cur, wq0)
            if b + 1 < BLOC:
                qt_cur = _load_qt(nc, g, b + 1)  # prefetch during B/C/D
            kt_next = None
            for th in range(2):
                pt_sb = _phase_b(nc, g, b, th, kt_cur)
                if th == 1 and b + 1 < BLOC:
                    # kt(b) slot free after B1; first chunks move in D/A window
                    kt_next = _load_kt_first(nc, g, b + 1)
                attnt = _phase_c(nc, g, b, th, pt_sb)
                if th == 0:
                    wq0 = _prefetch_wq(nc, g) if b + 1 < BLOC else None
                _phase_d(nc, g, b, th, attnt)
            if kt_next is not None:
                _load_kt_rest(nc, g, b + 1, kt_next)
                kt_cur = kt_next

    nc.compile()
    return nc


def _round_f32r(x):
    """Round float32 to the f32r grid (11 stored mantissa bits, RNE)."""
    b = np.ascontiguousarray(x, np.float32).view(np.uint32).astype(np.uint64)
    r = (b + 0x7FF + ((b >> 12) & 1)) & ~np.uint64(0xFFF)
    return r.astype(np.uint32).view(np.float32).reshape(x.shape)


def kernel(Q, K, V, Wq, bq, Wc, bc):
    global _LAST_IN_MAPS
    Q = np.asarray(Q, np.float32)
    K = np.asarray(K, np.float32)
    V = np.asarray(V, np.float32)
    Wq = np.asarray(Wq, np.float32)
    bq = np.asarray(bq, np.float32)
    Wc = np.asarray(Wc, np.float32)
    bc = np.asarray(bc, np.float32)

    if "nc" not in _NC_CACHE:
        _NC_CACHE["nc"] = _build()
    nc = _NC_CACHE["nc"]

    QT = _round_f32r(np.ascontiguousarray(Q.transpose(0, 2, 1)))
    KT_h = _round_f32r(np.ascontiguousarray(K.transpose(0, 2, 1)))
    Vbf = V.astype(ml_dtypes.bfloat16)
    QTbf = np.ascontiguousarray(Q.transpose(0, 2, 1)).astype(ml_dtypes.bfloat16)
    WqT = _round_f32r(np.ascontiguousarray(Wq.T))
    WcT = np.ascontiguousarray(Wc.T).astype(ml_dtypes.bfloat16)
    bc2 = bc.reshape(1, DOUT)

    in_maps = []
    for c in range(NCORES):
        s = slice(c * BLOC, (c + 1) * BLOC)
        in_maps.append(
            {
                "QT": QT[s],
                "KTd": KT_h[s],
                "Vbf": Vbf[s],
                "QTbf": QTbf[s],
                "WqT": WqT,
                "bq": bq,
                "WcT": WcT,
                "bc": bc2,
            }
        )

    _LAST_IN_MAPS = in_maps
    res = bass_utils.run_bass_kernel_spmd(nc, in_maps, core_ids=list(range(NCORES)))
    out = np.concatenate([res.results[c]["out"] for c in range(NCORES)], axis=0)
    attn = np.concatenate([res.results[c]["attnW"] for c in range(NCORES)], axis=0)
    return out, attn


# revision 41
# speedup vs baseline: 1.0372x; 1.0372x over previous
"""Trainium2 Bass kernel for nn_Attn_9637906612873.

Reference computation (per batch b of 32):
    Qm     = Q @ Wq.T + bq            # [TQ, DK]
    S      = Qm @ K.T                  # [TQ, L]
    P      = softmax(S, axis=-1)       # [TQ, L]  (returned as attn_weights)
    A      = P @ V                     # [TQ, DV]
    out    = concat([Q, A]) @ Wc.T + bc  # [TQ, DOUT]

Strategy: data-parallel over batch across 8 NeuronCores (4 batches/core),
weights replicated. On each core, per batch:
  phase A: Qm^T = Wq^T-chunks.T @ Q^T   (float32r, PSUM accum over q)
  phase B: S-rows per 128-row t-tile via Qm^T-chunks.T @ K^T (float32r),
           softmax on PSUM rows (DVE max, ACT exp+rowsum, DVE normalize),
           store P (f32) to attn output and P (bf16) to a DRAM scratch
  phase C: A^T = V-chunks.T @ P^T  (bf16; P^T streamed back from the
           scratch with DMA-transpose)
  phase D: out = [Q;A]^T-chunks.T @ Wc^T + bc  (bf16)

float32r = fp32 with the mantissa rounded to 11 stored bits; the PE runs it
at ~65 TF/s (vs 19 for fp32) with exact f32 accumulation, keeping softmax
logits accurate to ~4e-3 absolute. Host pre-rounds the QK-path operands so
DMA can feed float32r tiles directly.
"""
import sys

sys.path.insert(0, "/opt/trn_rl_repo")

from contextlib import ExitStack

import numpy as np
import ml_dtypes

import concourse.bass as bass
import concourse.tile as tile
from concourse import bacc, mybir
from concourse import bass_utils
from concourse.masks import make_identity

P = 128
B, TQ, L = 32, 1024, 2048
DQ, DK, DV, DOUT = 1024, 1024, 1024, 1024
NCORES = 8
BLOC = B // NCORES

f32 = mybir.dt.float32
f32r = mybir.dt.float32r
bf16 = mybir.dt.bfloat16

KC = DQ // P     # q chunks (mm1 contraction)
KT = DK // P     # k tiles
TT = TQ // P     # t tiles
LB = L // 512    # l blocks of 512
LC = L // P      # l chunks of 128
VT = DV // P     # v tiles
CC = (DQ + DV) // P  # combined chunks
OC = DOUT // 512  # out column blocks

_NC_CACHE = {}
_LAST_IN_MAPS = None


def _load_qt(nc, g, b):
    qt_sb = g["qt_pool"].tile([P, KC, TQ], f32r, name="qt_sb")
    src = g["QT"][b].rearrange("(qc p) t -> p qc t", p=P)
    for h in range(2):
        nc.gpsimd.dma_start(out=qt_sb[:, h * 4 : (h + 1) * 4, :], in_=src[:, h * 4 : (h + 1) * 4, :])
    return qt_sb


def _load_kt_first(nc, g, b):
    kt_sb = g["kt_pool"].tile([P, KC, L], f32r, name="kt_sb")
    src = g["KTd"][b].rearrange("(kc p) l -> p kc l", p=P)
    for h in range(2):
        nc.gpsimd.dma_start(out=kt_sb[:, h * 2 : (h + 1) * 2, :], in_=src[:, h * 2 : (h + 1) * 2, :])
    return kt_sb


def _load_kt_rest(nc, g, b, kt_sb):
    src = g["KTd"][b].rearrange("(kc p) l -> p kc l", p=P)
    for h in range(2, 4):
        nc.gpsimd.dma_start(out=kt_sb[:, h * 2 : (h + 1) * 2, :], in_=src[:, h * 2 : (h + 1) * 2, :])


def _prefetch_wq(nc, g):
    """First k-tile's Wq chunks, loaded ahead so phase A starts immediately."""
    tiles = []
    wq_src = g["WqT"][:, 0:P].rearrange("(qc p) k -> p qc k", p=P)
    for qh in range(2):
        w = g["wq_pool"].tile([P, KC // 2, P], f32r, name="wq_sb")
        nc.scalar.dma_start(out=w, in_=wq_src[:, qh * 4 : (qh + 1) * 4, :])
        tiles.append(w)
    return tiles


def _phase_a(nc, g, b, qt_sb, wq0):
    """QmT[k, t] = sum_q WqT[q,k] * QT[q,t] + bq  (float32r)."""
    qmt_sb = g["qmt_pool"].tile([P, KT, TQ], f32r, name="qmt_sb")
    for kt in range(KT):
        wq_src = g["WqT"][:, kt * P : (kt + 1) * P].rearrange("(qc p) k -> p qc k", p=P)
        pa = g["ps"].tile([P, 2, 512], f32, tag="ps", name="pa")
        for qh in range(2):
            if kt == 0 and wq0 is not None:
                wq_sb = wq0[qh]
            else:
                wq_sb = g["wq_pool"].tile([P, KC // 2, P], f32r, name="wq_sb")
                nc.scalar.dma_start(
                    out=wq_sb, in_=wq_src[:, qh * 4 : (qh + 1) * 4, :]
                )
            for q4 in range(KC // 2):
                qc = qh * 4 + q4
                for th in range(2):
                    nc.tensor.matmul(
                        pa[:, th, :],
                        wq_sb[:, q4, :],
                        qt_sb[:, qc, th * 512 : (th + 1) * 512],
                        start=(qc == 0),
                        stop=(qc == KC - 1),
                    )
        nc.vector.tensor_scalar_add(
            qmt_sb[:, kt, :],
            pa.rearrange("p a b -> p (a b)"),
            g["bq_sb"][:, kt : kt + 1],
        )
    g["qmt_sb"] = qmt_sb


def _transpose_pb(nc, g, pb_sb, col, pt_sb):
    """16 PE transposes of one P t-tile (bf16) into PT[l, t] columns."""
    tp = g["ps"].tile([P, LC, P], bf16, tag="ps", name="tp")
    for lc in range(LC):
        nc.tensor.transpose(
            tp[:, lc, :], pb_sb[:, lc * P : (lc + 1) * P], g["ident"][:]
        )
    nc.scalar.activation(
        pt_sb[:, :, col * P : (col + 1) * P],
        tp[:, :, :],
        mybir.ActivationFunctionType.Copy,
    )


def _phase_b(nc, g, b, th, kt_sb):
    """scores (float32r) + softmax per t-tile; PE-transpose P into PT (bf16)."""
    pt_sb = g["pt_pool"].tile([P, LC, 512], bf16, name="pt_sb")
    pending = None
    for tt in range(th * 4, th * 4 + 4):
        pb_ps = g["ps"].tile([P, LB, 512], f32, tag="ps", name="pb_ps")
        for kc in range(KC):
            lhsT = g["qmt_sb"][:, kc, tt * P : (tt + 1) * P]
            for lb in range(LB):
                nc.tensor.matmul(
                    pb_ps[:, lb, :],
                    lhsT,
                    kt_sb[:, kc, lb * 512 : (lb + 1) * 512],
                    start=(kc == 0),
                    stop=(kc == KC - 1),
                )
        if pending is not None:
            _transpose_pb(nc, g, pending[0], pending[1], pt_sb)
        flat = pb_ps.rearrange("p a b -> p (a b)")
        neg_m = g["small"].tile([P, 1], f32, name="neg_m")
        nc.vector.reduce_max(neg_m, flat, axis=mybir.AxisListType.X, negate=True)
        e_sb = g["e_pool"].tile([P, L], f32, name="e_sb")
        s_sum = g["small"].tile([P, 1], f32, name="s_sum")
        nc.scalar.activation(
            e_sb[:],
            flat,
            mybir.ActivationFunctionType.Exp,
            bias=neg_m[:],
            accum_out=s_sum[:],
        )
        r_sb = g["small"].tile([P, 1], f32, name="r_sb")
        nc.vector.reciprocal(r_sb, s_sum)
        # critical path to phase C: normalized bf16 copy straight from ACT
        pb_sb = g["pb_pool"].tile([P, L], bf16, name="pb_sb")
        nc.scalar.activation(
            pb_sb[:], e_sb[:], mybir.ActivationFunctionType.Copy, scale=r_sb[:]
        )
        pending = (pb_sb, tt - th * 4)
        # attnW output (f32), off the critical path
        nc.vector.tensor_scalar_mul(e_sb[:], e_sb[:], r_sb[:])
        nc.sync.dma_start(out=g["attnW"][b, tt * P : (tt + 1) * P, :], in_=e_sb)
    _transpose_pb(nc, g, pending[0], pending[1], pt_sb)
    return pt_sb


def _phase_c(nc, g, b, th, pt_sb):
    """attnT[v, t-half] = sum_l V[l,v] * P[t,l]  (bf16, PT resident in SBUF)."""
    attnt_sb = g["attnt_pool"].tile([P, VT, 512], bf16, name="attnt_sb")
    pc0 = g["ps"].tile([P, 4, 512], f32, tag="ps", name="pc0")
    pc1 = g["ps"].tile([P, 4, 512], f32, tag="ps", name="pc1")
    pcs = [pc0, pc1]
    for lc in range(LC):
        v_sb = g["v_pool"].tile([P, DV], bf16, name="v_sb")
        nc.gpsimd.dma_start(
            out=v_sb, in_=g["Vbf"][b, lc * P : (lc + 1) * P, :]
        )
        for vt in range(VT):
            nc.tensor.matmul(
                pcs[vt // 4][:, vt % 4, :],
                v_sb[:, vt * P : (vt + 1) * P],
                pt_sb[:, lc, :],
                start=(lc == 0),
                stop=(lc == LC - 1),
            )
    for vt in range(VT):
        nc.scalar.activation(
            attnt_sb[:, vt, :],
            pcs[vt // 4][:, vt % 4, :],
            mybir.ActivationFunctionType.Copy,
        )
    return attnt_sb


def _phase_d(nc, g, b, th, attnt_sb):
    """out[t-half, o] = sum_c comb[c,t] * WcT[c,o] + bc  (bf16)."""
    for tq in [th]:
        pd0 = g["ps"].tile([P, 4, 512], f32, tag="ps", name="pd0")
        pd1 = g["ps"].tile([P, 4, 512], f32, tag="ps", name="pd1")
        pds = [pd0, pd1]
        for cb in range(CC // 2):
            wc_sb = g["wc_pool"].tile([P, 2, DOUT], bf16, name="wc_sb")
            nc.scalar.dma_start(
                out=wc_sb,
                in_=g["WcT"][cb * 2 * P : (cb + 1) * 2 * P, :].rearrange(
                    "(c p) o -> p c o", p=P
                ),
            )
            qq = None
            if cb * 2 < KC:
                qq = g["qtbf_pool"].tile([P, 2, 512], bf16, name="qq")
                nc.scalar.dma_start(
                    out=qq,
                    in_=g["QTbf"][
                        b, cb * 2 * P : (cb + 1) * 2 * P, tq * 512 : (tq + 1) * 512
                    ].rearrange("(c p) t -> p c t", p=P),
                )
            for cc in range(2):
                c = cb * 2 + cc
                for i in range(4):
                    tt = tq * 4 + i
                    if c < KC:
                        lhsT = qq[:, cc, i * P : (i + 1) * P]
                    else:
                        lhsT = attnt_sb[:, c - KC, i * P : (i + 1) * P]
                    for oc in range(OC):
                        j = i * OC + oc
                        nc.tensor.matmul(
                            pds[j // 4][:, j % 4, :],
                            lhsT,
                            wc_sb[:, cc, oc * 512 : (oc + 1) * 512],
                            start=(c == 0),
                            stop=(c == CC - 1),
                        )
        for i in range(4):
            tt = tq * 4 + i
            for oc in range(OC):
                j = i * OC + oc
                o_sb = g["out_pool"].tile([P, 512], f32, name="o_sb")
                nc.vector.tensor_add(
                    o_sb[:],
                    pds[j // 4][:, j % 4, :],
                    g["bc_sb"][:, oc * 512 : (oc + 1) * 512],
                )
                nc.sync.dma_start(
                    out=g["out"][b, tt * P : (tt + 1) * P, oc * 512 : (oc + 1) * 512],
                    in_=o_sb,
                )


def _build():
    nc = bacc.Bacc(
        "TRN2",
        target_bir_lowering=False,
        debug=False,
        enable_asserts=True,
        num_devices=NCORES,
    )
    g = {}
    g["QT"] = nc.dram_tensor("QT", [BLOC, DQ, TQ], f32r, kind="ExternalInput").ap()
    g["KTd"] = nc.dram_tensor("KTd", [BLOC, DK, L], f32r, kind="ExternalInput").ap()
    g["Vbf"] = nc.dram_tensor("Vbf", [BLOC, L, DV], bf16, kind="ExternalInput").ap()
    g["QTbf"] = nc.dram_tensor("QTbf", [BLOC, DQ, TQ], bf16, kind="ExternalInput").ap()
    g["WqT"] = nc.dram_tensor("WqT", [DQ, DK], f32r, kind="ExternalInput").ap()
    g["bq"] = nc.dram_tensor("bq", [DK], f32, kind="ExternalInput").ap()
    g["WcT"] = nc.dram_tensor("WcT", [DQ + DV, DOUT], bf16, kind="ExternalInput").ap()
    g["bc"] = nc.dram_tensor("bc", [1, DOUT], f32, kind="ExternalInput").ap()
    g["attnW"] = nc.dram_tensor(
        "attnW", [BLOC, TQ, L], f32, kind="ExternalOutput"
    ).ap()
    g["out"] = nc.dram_tensor(
        "out", [BLOC, TQ, DOUT], f32, kind="ExternalOutput"
    ).ap()

    with tile.TileContext(nc) as tc, ExitStack() as ctx:
        for name, bufs, space in [
            ("const", 1, "SBUF"),
            ("qt_pool", 1, "SBUF"),
            ("kt_pool", 1, "SBUF"),
            ("qmt_pool", 1, "SBUF"),
            ("attnt_pool", 1, "SBUF"),
            ("wq_pool", 3, "SBUF"),
            ("e_pool", 2, "SBUF"),
            ("pb_pool", 2, "SBUF"),
            ("pt_pool", 1, "SBUF"),
            ("v_pool", 3, "SBUF"),
            ("qtbf_pool", 2, "SBUF"),
            ("wc_pool", 2, "SBUF"),
            ("out_pool", 1, "SBUF"),
            ("small", 4, "SBUF"),
            ("ps", 2, "PSUM"),
            ("dram", 2, "DRAM"),
        ]:
            g[name] = ctx.enter_context(tc.tile_pool(name=name, bufs=bufs, space=space))

        bq_sb = g["const"].tile([P, KT], f32)
        nc.sync.dma_start(out=bq_sb, in_=g["bq"].rearrange("(kt p) -> p kt", p=P))
        bc_sb = g["const"].tile([P, DOUT], f32)
        nc.sync.dma_start(out=bc_sb, in_=g["bc"].to_broadcast([P, DOUT]))
        ident = g["const"].tile([P, P], bf16)
        make_identity(nc, ident)
        g["bq_sb"] = bq_sb
        g["bc_sb"] = bc_sb
        g["ident"] = ident

        qt_cur = _load_qt(nc, g, 0)
        kt_cur = _load_kt_first(nc, g, 0)
        _load_kt_rest(nc, g, 0, kt_cur)
        wq0 = None
        for b in range(BLOC):
            _phase_a(nc, g, b, qt_cur, wq0)
            if b + 1 < BLOC:
                qt_cur = _load_qt(nc, g, b + 1)  # prefetch during B/C/D
            kt_next = None
            for th in range(2):
                pt_sb = _phase_b(nc, g, b, th, kt_cur)
                if th == 1 and b + 1 < BLOC:
                    # kt(b) slot free after B1; first chunks move in D/A window
                    kt_next = _load_kt_first(nc, g, b + 1)
                attnt = _phase_c(nc, g, b, th, pt_sb)
                if th == 0:
                    wq0 = _prefetch_wq(nc, g) if b + 1 < BLOC else None
                _phase_d(nc, g, b, th, attnt)
            if kt_next is not None:
                _load_kt_rest(nc, g, b + 1, kt_next)
                kt_cur = kt_next

    nc.compile()
    return nc


def _round_f32r(x):
    """Round float32 to the f32r grid (11 stored mantissa bits, RNE)."""
    b = np.ascontiguousarray(x, np.float32).view(np.uint32).astype(np.uint64)
    r = (b + 0x7FF + ((b >> 12) & 1)) & ~np.uint64(0xFFF)
    return r.astype(np.uint32).view(np.float32).reshape(x.shape)


def kernel(Q, K, V, Wq, bq, Wc, bc):
    global _LAST_IN_MAPS
    Q = np.asarray(Q, np.float32)
    K = np.asarray(K, np.float32)
    V = np.asarray(V, np.float32)
    Wq = np.asarray(Wq, np.float32)
    bq = np.asarray(bq, np.float32)
    Wc = np.asarray(Wc, np.float32)
    bc = np.asarray(bc, np.float32)

    if "nc" not in _NC_CACHE:
        _NC_CACHE["nc"] = _build()
    nc = _NC_CACHE["nc"]

    QT = _round_f32r(np.ascontiguousarray(Q.transpose(0, 2, 1)))
    KT_h = _round_f32r(np.ascontiguousarray(K.transpose(0, 2, 1)))
    Vbf = V.astype(ml_dtypes.bfloat16)
    QTbf = np.ascontiguousarray(Q.transpose(0, 2, 1)).astype(ml_dtypes.bfloat16)
    WqT = _round_f32r(np.ascontiguousarray(Wq.T))
    WcT = np.ascontiguousarray(Wc.T).astype(ml_dtypes.bfloat16)
    bc2 = bc.reshape(1, DOUT)

    in_maps = []
    for c in range(NCORES):
        s = slice(c * BLOC, (c + 1) * BLOC)
        in_maps.append(
            {
                "QT": QT[s],
                "KTd": KT_h[s],
                "Vbf": Vbf[s],
                "QTbf": QTbf[s],
                "WqT": WqT,
                "bq": bq,
                "WcT": WcT,
                "bc": bc2,
            }
        )

    _LAST_IN_MAPS = in_maps
    res = bass_utils.run_bass_kernel_spmd(nc, in_maps, core_ids=list(range(NCORES)))
    out = np.concatenate([res.results[c]["out"] for c in range(NCORES)], axis=0)
    attn = np.concatenate([res.results[c]["attnW"] for c in range(NCORES)], axis=0)
    return out, attn


# revision 43
# speedup vs baseline: 1.0477x; 1.0101x over previous
"""Trainium2 Bass kernel for nn_Attn_9637906612873.

Reference computation (per batch b of 32):
    Qm     = Q @ Wq.T + bq            # [TQ, DK]
    S      = Qm @ K.T                  # [TQ, L]
    P      = softmax(S, axis=-1)       # [TQ, L]  (returned as attn_weights)
    A      = P @ V                     # [TQ, DV]
    out    = concat([Q, A]) @ Wc.T + bc  # [TQ, DOUT]

Strategy: data-parallel over batch across 8 NeuronCores (4 batches/core),
weights replicated. On each core, per batch:
  phase A: Qm^T = Wq^T-chunks.T @ Q^T   (float32r, PSUM accum over q)
  phase B: S-rows per 128-row t-tile via Qm^T-chunks.T @ K^T (float32r),
           softmax on PSUM rows (DVE max, ACT exp+rowsum, DVE normalize),
           store P (f32) to attn output and P (bf16) to a DRAM scratch
  phase C: A^T = V-chunks.T @ P^T  (bf16; P^T streamed back from the
           scratch with DMA-transpose)
  phase D: out = [Q;A]^T-chunks.T @ Wc^T + bc  (bf16)

float32r = fp32 with the mantissa rounded to 11 stored bits; the PE runs it
at ~65 TF/s (vs 19 for fp32) with exact f32 accumulation, keeping softmax
logits accurate to ~4e-3 absolute. Host pre-rounds the QK-path operands so
DMA can feed float32r tiles directly.
"""
import sys

sys.path.insert(0, "/opt/trn_rl_repo")

from contextlib import ExitStack

import numpy as np
import ml_dtypes

import concourse.bass as bass
import concourse.tile as tile
from concourse import bacc, mybir
from concourse import bass_utils
from concourse.masks import make_identity

P = 128
B, TQ, L = 32, 1024, 2048
DQ, DK, DV, DOUT = 1024, 1024, 1024, 1024
NCORES = 8
BLOC = B // NCORES

f32 = mybir.dt.float32
f32r = mybir.dt.float32r
bf16 = mybir.dt.bfloat16

KC = DQ // P     # q chunks (mm1 contraction)
KT = DK // P     # k tiles
TT = TQ // P     # t tiles
LB = L // 512    # l blocks of 512
LC = L // P      # l chunks of 128
VT = DV // P     # v tiles
CC = (DQ + DV) // P  # combined chunks
OC = DOUT // 512  # out column blocks

_NC_CACHE = {}
_LAST_IN_MAPS = None


def _load_qt(nc, g, b):
    qt_sb = g["qt_pool"].tile([P, KC, TQ], f32r, name="qt_sb")
    src = g["QT"][b].rearrange("(qc p) t -> p qc t", p=P)
    for h in range(2):
        nc.gpsimd.dma_start(out=qt_sb[:, h * 4 : (h + 1) * 4, :], in_=src[:, h * 4 : (h + 1) * 4, :])
    return qt_sb


def _load_kt_first(nc, g, b):
    kt_sb = g["kt_pool"].tile([P, KC, L], f32r, name="kt_sb")
    src = g["KTd"][b].rearrange("(kc p) l -> p kc l", p=P)
    for h in range(2):
        nc.gpsimd.dma_start(out=kt_sb[:, h * 2 : (h + 1) * 2, :], in_=src[:, h * 2 : (h + 1) * 2, :])
    return kt_sb


def _load_kt_rest(nc, g, b, kt_sb):
    src = g["KTd"][b].rearrange("(kc p) l -> p kc l", p=P)
    for h in range(2, 4):
        nc.gpsimd.dma_start(out=kt_sb[:, h * 2 : (h + 1) * 2, :], in_=src[:, h * 2 : (h + 1) * 2, :])


def _prefetch_wq(nc, g):
    """First k-tile's Wq chunks, loaded ahead so phase A starts immediately."""
    tiles = []
    wq_src = g["WqT"][:, 0:P].rearrange("(qc p) k -> p qc k", p=P)
    for qh in range(2):
        w = g["wq_pool"].tile([P, KC // 2, P], f32r, name="wq_sb")
        nc.scalar.dma_start(out=w, in_=wq_src[:, qh * 4 : (qh + 1) * 4, :])
        tiles.append(w)
    return tiles


def _phase_a(nc, g, b, qt_sb, wq0):
    """QmT[k, t] = sum_q WqT[q,k] * QT[q,t] + bq  (float32r)."""
    qmt_sb = g["qmt_pool"].tile([P, KT, TQ], f32r, name="qmt_sb")
    for kt in range(KT):
        wq_src = g["WqT"][:, kt * P : (kt + 1) * P].rearrange("(qc p) k -> p qc k", p=P)
        pa = g["ps"].tile([P, 2, 512], f32, tag="ps", name="pa")
        for qh in range(2):
            if kt == 0 and wq0 is not None:
                wq_sb = wq0[qh]
            else:
                wq_sb = g["wq_pool"].tile([P, KC // 2, P], f32r, name="wq_sb")
                nc.scalar.dma_start(
                    out=wq_sb, in_=wq_src[:, qh * 4 : (qh + 1) * 4, :]
                )
            for q4 in range(KC // 2):
                qc = qh * 4 + q4
                for th in range(2):
                    nc.tensor.matmul(
                        pa[:, th, :],
                        wq_sb[:, q4, :],
                        qt_sb[:, qc, th * 512 : (th + 1) * 512],
                        start=(qc == 0),
                        stop=(qc == KC - 1),
                    )
        nc.vector.tensor_scalar_add(
            qmt_sb[:, kt, :],
            pa.rearrange("p a b -> p (a b)"),
            g["bq_sb"][:, kt : kt + 1],
        )
    g["qmt_sb"] = qmt_sb


def _transpose_pb(nc, g, pb_sb, col, pt_sb):
    """16 PE transposes of one P t-tile (bf16) into PT[l, t] columns."""
    tp = g["ps"].tile([P, LC, P], bf16, tag="ps", name="tp")
    for lc in range(LC):
        nc.tensor.transpose(
            tp[:, lc, :], pb_sb[:, lc * P : (lc + 1) * P], g["ident"][:]
        )
    nc.scalar.activation(
        pt_sb[:, :, col * P : (col + 1) * P],
        tp[:, :, :],
        mybir.ActivationFunctionType.Copy,
    )


def _phase_b(nc, g, b, th, kt_sb):
    """scores (float32r) + softmax per t-tile; PE-transpose P into PT (bf16)."""
    pt_sb = g["pt_pool"].tile([P, LC, 512], bf16, name="pt_sb")
    pending = None
    for tt in range(th * 4, th * 4 + 4):
        pb_ps = g["ps"].tile([P, LB, 512], f32, tag="ps", name="pb_ps")
        for kc in range(KC):
            lhsT = g["qmt_sb"][:, kc, tt * P : (tt + 1) * P]
            for lb in range(LB):
                nc.tensor.matmul(
                    pb_ps[:, lb, :],
                    lhsT,
                    kt_sb[:, kc, lb * 512 : (lb + 1) * 512],
                    start=(kc == 0),
                    stop=(kc == KC - 1),
                )
        if pending is not None:
            _transpose_pb(nc, g, pending[0], pending[1], pt_sb)
        flat = pb_ps.rearrange("p a b -> p (a b)")
        neg_m = g["small"].tile([P, 1], f32, name="neg_m")
        nc.vector.reduce_max(neg_m, flat, axis=mybir.AxisListType.X, negate=True)
        e_sb = g["e_pool"].tile([P, L], f32, name="e_sb")
        s_sum = g["small"].tile([P, 1], f32, name="s_sum")
        nc.scalar.activation(
            e_sb[:],
            flat,
            mybir.ActivationFunctionType.Exp,
            bias=neg_m[:],
            accum_out=s_sum[:],
        )
        r_sb = g["small"].tile([P, 1], f32, name="r_sb")
        nc.vector.reciprocal(r_sb, s_sum)
        # critical path to phase C: normalized bf16 copy straight from ACT
        pb_sb = g["pb_pool"].tile([P, L], bf16, name="pb_sb")
        nc.scalar.activation(
            pb_sb[:], e_sb[:], mybir.ActivationFunctionType.Copy, scale=r_sb[:]
        )
        pending = (pb_sb, tt - th * 4)
        # attnW output (f32), off the critical path
        nc.vector.tensor_scalar_mul(e_sb[:], e_sb[:], r_sb[:])
        nc.sync.dma_start(out=g["attnW"][b, tt * P : (tt + 1) * P, :], in_=e_sb)
    _transpose_pb(nc, g, pending[0], pending[1], pt_sb)
    return pt_sb


def _phase_c(nc, g, b, th, pt_sb):
    """attnT[v, t-half] = sum_l V[l,v] * P[t,l]  (bf16, PT resident in SBUF)."""
    attnt_sb = g["attnt_pool"].tile([P, VT, 512], bf16, name="attnt_sb")
    pc0 = g["ps"].tile([P, 4, 512], f32, tag="ps", name="pc0")
    pc1 = g["ps"].tile([P, 4, 512], f32, tag="ps", name="pc1")
    pcs = [pc0, pc1]
    for lc in range(LC):
        v_sb = g["v_pool"].tile([P, DV], bf16, name="v_sb")
        nc.gpsimd.dma_start(
            out=v_sb, in_=g["Vbf"][b, lc * P : (lc + 1) * P, :]
        )
        for vt in range(VT):
            nc.tensor.matmul(
                pcs[vt // 4][:, vt % 4, :],
                v_sb[:, vt * P : (vt + 1) * P],
                pt_sb[:, lc, :],
                start=(lc == 0),
                stop=(lc == LC - 1),
            )
    for vt in range(VT):
        nc.scalar.activation(
            attnt_sb[:, vt, :],
            pcs[vt // 4][:, vt % 4, :],
            mybir.ActivationFunctionType.Copy,
        )
    return attnt_sb


def _phase_d(nc, g, b, th, attnt_sb):
    """out[t-half, o] = sum_c comb[c,t] * WcT[c,o] + bc  (bf16)."""
    for tq in [th]:
        pd0 = g["ps"].tile([P, 4, 512], f32, tag="ps", name="pd0")
        pd1 = g["ps"].tile([P, 4, 512], f32, tag="ps", name="pd1")
        pds = [pd0, pd1]
        for cb in range(CC // 2):
            wc_sb = g["wc_pool"].tile([P, 2, DOUT], bf16, name="wc_sb")
            nc.scalar.dma_start(
                out=wc_sb,
                in_=g["WcT"][cb * 2 * P : (cb + 1) * 2 * P, :].rearrange(
                    "(c p) o -> p c o", p=P
                ),
            )
            qq = None
            if cb * 2 < KC:
                qq = g["qtbf_pool"].tile([P, 2, 512], bf16, name="qq")
                nc.scalar.dma_start(
                    out=qq,
                    in_=g["QTbf"][
                        b, cb * 2 * P : (cb + 1) * 2 * P, tq * 512 : (tq + 1) * 512
                    ].rearrange("(c p) t -> p c t", p=P),
                )
            for cc in range(2):
                c = cb * 2 + cc
                for i in range(4):
                    tt = tq * 4 + i
                    if c < KC:
                        lhsT = qq[:, cc, i * P : (i + 1) * P]
                    else:
                        lhsT = attnt_sb[:, c - KC, i * P : (i + 1) * P]
                    for oc in range(OC):
                        j = i * OC + oc
                        nc.tensor.matmul(
                            pds[j // 4][:, j % 4, :],
                            lhsT,
                            wc_sb[:, cc, oc * 512 : (oc + 1) * 512],
                            start=(c == 0),
                            stop=(c == CC - 1),
                        )
        for i in range(4):
            tt = tq * 4 + i
            for oc in range(OC):
                j = i * OC + oc
                o_sb = g["out_pool"].tile([P, 512], f32, name="o_sb")
                nc.vector.tensor_add(
                    o_sb[:],
                    pds[j // 4][:, j % 4, :],
                    g["bc_sb"][:, oc * 512 : (oc + 1) * 512],
                )
                nc.sync.dma_start(
                    out=g["out"][b, tt * P : (tt + 1) * P, oc * 512 : (oc + 1) * 512],
                    in_=o_sb,
                )


def _build():
    nc = bacc.Bacc(
        "TRN2",
        target_bir_lowering=False,
        debug=False,
        enable_asserts=True,
        num_devices=NCORES,
    )
    g = {}
    g["QT"] = nc.dram_tensor("QT", [BLOC, DQ, TQ], f32r, kind="ExternalInput").ap()
    g["KTd"] = nc.dram_tensor("KTd", [BLOC, DK, L], f32r, kind="ExternalInput").ap()
    g["Vbf"] = nc.dram_tensor("Vbf", [BLOC, L, DV], bf16, kind="ExternalInput").ap()
    g["QTbf"] = nc.dram_tensor("QTbf", [BLOC, DQ, TQ], bf16, kind="ExternalInput").ap()
    g["WqT"] = nc.dram_tensor("WqT", [DQ, DK], f32r, kind="ExternalInput").ap()
    g["bq"] = nc.dram_tensor("bq", [DK], f32, kind="ExternalInput").ap()
    g["WcT"] = nc.dram_tensor("WcT", [DQ + DV, DOUT], bf16, kind="ExternalInput").ap()
    g["bc"] = nc.dram_tensor("bc", [1, DOUT], bf16, kind="ExternalInput").ap()
    g["attnW"] = nc.dram_tensor(
        "attnW", [BLOC, TQ, L], f32, kind="ExternalOutput"
    ).ap()
    g["out"] = nc.dram_tensor(
        "out", [BLOC, TQ, DOUT], f32, kind="ExternalOutput"
    ).ap()

    with tile.TileContext(nc) as tc, ExitStack() as ctx:
        for name, bufs, space in [
            ("const", 1, "SBUF"),
            ("qt_pool", 1, "SBUF"),
            ("kt_pool", 1, "SBUF"),
            ("qmt_pool", 1, "SBUF"),
            ("attnt_pool", 1, "SBUF"),
            ("wq_pool", 3, "SBUF"),
            ("e_pool", 2, "SBUF"),
            ("pb_pool", 2, "SBUF"),
            ("pt_pool", 1, "SBUF"),
            ("v_pool", 2, "SBUF"),
            ("qtbf_pool", 2, "SBUF"),
            ("wc_pool", 2, "SBUF"),
            ("out_pool", 3, "SBUF"),
            ("small", 4, "SBUF"),
            ("ps", 2, "PSUM"),
            ("dram", 2, "DRAM"),
        ]:
            g[name] = ctx.enter_context(tc.tile_pool(name=name, bufs=bufs, space=space))

        bq_sb = g["const"].tile([P, KT], f32)
        nc.sync.dma_start(out=bq_sb, in_=g["bq"].rearrange("(kt p) -> p kt", p=P))
        bc_sb = g["const"].tile([P, DOUT], bf16, name="bc_sb")
        nc.sync.dma_start(out=bc_sb, in_=g["bc"].to_broadcast([P, DOUT]))
        ident = g["const"].tile([P, P], bf16)
        make_identity(nc, ident)
        g["bq_sb"] = bq_sb
        g["bc_sb"] = bc_sb
        g["ident"] = ident

        qt_cur = _load_qt(nc, g, 0)
        kt_cur = _load_kt_first(nc, g, 0)
        _load_kt_rest(nc, g, 0, kt_cur)
        wq0 = None
        for b in range(BLOC):
            _phase_a(nc, g, b, qt_cur, wq0)
            if b + 1 < BLOC:
                qt_cur = _load_qt(nc, g, b + 1)  # prefetch during B/C/D
            kt_next = None
            for th in range(2):
                pt_sb = _phase_b(nc, g, b, th, kt_cur)
                if th == 1 and b + 1 < BLOC:
                    # kt(b) slot free after B1; first chunks move in D/A window
                    kt_next = _load_kt_first(nc, g, b + 1)
                attnt = _phase_c(nc, g, b, th, pt_sb)
                if th == 0:
                    wq0 = _prefetch_wq(nc, g) if b + 1 < BLOC else None
                _phase_d(nc, g, b, th, attnt)
            if kt_next is not None:
                _load_kt_rest(nc, g, b + 1, kt_next)
                kt_cur = kt_next

    nc.compile()
    return nc


def _round_f32r(x):
    """Round float32 to the f32r grid (11 stored mantissa bits, RNE)."""
    b = np.ascontiguousarray(x, np.float32).view(np.uint32).astype(np.uint64)
    r = (b + 0x7FF + ((b >> 12) & 1)) & ~np.uint64(0xFFF)
    return r.astype(np.uint32).view(np.float32).reshape(x.shape)


def kernel(Q, K, V, Wq, bq, Wc, bc):
    global _LAST_IN_MAPS
    Q = np.asarray(Q, np.float32)
    K = np.asarray(K, np.float32)
    V = np.asarray(V, np.float32)
    Wq = np.asarray(Wq, np.float32)
    bq = np.asarray(bq, np.float32)
    Wc = np.asarray(Wc, np.float32)
    bc = np.asarray(bc, np.float32)

    if "nc" not in _NC_CACHE:
        _NC_CACHE["nc"] = _build()
    nc = _NC_CACHE["nc"]

    QT = _round_f32r(np.ascontiguousarray(Q.transpose(0, 2, 1)))
    KT_h = _round_f32r(np.ascontiguousarray(K.transpose(0, 2, 1)))
    Vbf = V.astype(ml_dtypes.bfloat16)
    QTbf = np.ascontiguousarray(Q.transpose(0, 2, 1)).astype(ml_dtypes.bfloat16)
    WqT = _round_f32r(np.ascontiguousarray(Wq.T))
    WcT = np.ascontiguousarray(Wc.T).astype(ml_dtypes.bfloat16)
    bc2 = bc.reshape(1, DOUT)

    in_maps = []
    for c in range(NCORES):
        s = slice(c * BLOC, (c + 1) * BLOC)
        in_maps.append(
            {
                "QT": QT[s],
                "KTd": KT_h[s],
                "Vbf": Vbf[s],
                "QTbf": QTbf[s],
                "WqT": WqT,
                "bq": bq,
                "WcT": WcT,
                "bc": bc2.astype(ml_dtypes.bfloat16),
            }
        )

    _LAST_IN_MAPS = in_maps
    res = bass_utils.run_bass_kernel_spmd(nc, in_maps, core_ids=list(range(NCORES)))
    out = np.concatenate([res.results[c]["out"] for c in range(NCORES)], axis=0)
    attn = np.concatenate([res.results[c]["attnW"] for c in range(NCORES)], axis=0)
    return out, attn


# revision 44
# speedup vs baseline: 1.0478x; 1.0001x over previous
"""Trainium2 Bass kernel for nn_Attn_9637906612873.

Reference computation (per batch b of 32):
    Qm     = Q @ Wq.T + bq            # [TQ, DK]
    S      = Qm @ K.T                  # [TQ, L]
    P      = softmax(S, axis=-1)       # [TQ, L]  (returned as attn_weights)
    A      = P @ V                     # [TQ, DV]
    out    = concat([Q, A]) @ Wc.T + bc  # [TQ, DOUT]

Strategy: data-parallel over batch across 8 NeuronCores (4 batches/core),
weights replicated. On each core, per batch:
  phase A: Qm^T = Wq^T-chunks.T @ Q^T   (float32r, PSUM accum over q)
  phase B: S-rows per 128-row t-tile via Qm^T-chunks.T @ K^T (float32r),
           softmax on PSUM rows (DVE max, ACT exp+rowsum, DVE normalize),
           store P (f32) to attn output and P (bf16) to a DRAM scratch
  phase C: A^T = V-chunks.T @ P^T  (bf16; P^T streamed back from the
           scratch with DMA-transpose)
  phase D: out = [Q;A]^T-chunks.T @ Wc^T + bc  (bf16)

float32r = fp32 with the mantissa rounded to 11 stored bits; the PE runs it
at ~65 TF/s (vs 19 for fp32) with exact f32 accumulation, keeping softmax
logits accurate to ~4e-3 absolute. Host pre-rounds the QK-path operands so
DMA can feed float32r tiles directly.
"""
import sys

sys.path.insert(0, "/opt/trn_rl_repo")

from contextlib import ExitStack

import numpy as np
import ml_dtypes

import concourse.bass as bass
import concourse.tile as tile
from concourse import bacc, mybir
from concourse import bass_utils
from concourse.masks import make_identity

P = 128
B, TQ, L = 32, 1024, 2048
DQ, DK, DV, DOUT = 1024, 1024, 1024, 1024
NCORES = 8
BLOC = B // NCORES

f32 = mybir.dt.float32
f32r = mybir.dt.float32r
bf16 = mybir.dt.bfloat16

KC = DQ // P     # q chunks (mm1 contraction)
KT = DK // P     # k tiles
TT = TQ // P     # t tiles
LB = L // 512    # l blocks of 512
LC = L // P      # l chunks of 128
VT = DV // P     # v tiles
CC = (DQ + DV) // P  # combined chunks
OC = DOUT // 512  # out column blocks

_NC_CACHE = {}
_LAST_IN_MAPS = None


def _load_qt(nc, g, b):
    qt_sb = g["qt_pool"].tile([P, KC, TQ], f32r, name="qt_sb")
    src = g["QT"][b].rearrange("(qc p) t -> p qc t", p=P)
    for h in range(2):
        nc.gpsimd.dma_start(out=qt_sb[:, h * 4 : (h + 1) * 4, :], in_=src[:, h * 4 : (h + 1) * 4, :])
    return qt_sb


def _load_kt_first(nc, g, b):
    kt_sb = g["kt_pool"].tile([P, KC, L], f32r, name="kt_sb")
    src = g["KTd"][b].rearrange("(kc p) l -> p kc l", p=P)
    for h in range(2):
        nc.gpsimd.dma_start(out=kt_sb[:, h * 2 : (h + 1) * 2, :], in_=src[:, h * 2 : (h + 1) * 2, :])
    return kt_sb


def _load_kt_rest(nc, g, b, kt_sb):
    src = g["KTd"][b].rearrange("(kc p) l -> p kc l", p=P)
    for h in range(2, 4):
        nc.gpsimd.dma_start(out=kt_sb[:, h * 2 : (h + 1) * 2, :], in_=src[:, h * 2 : (h + 1) * 2, :])


def _prefetch_wq(nc, g):
    """First k-tile's Wq chunks, loaded ahead so phase A starts immediately."""
    tiles = []
    wq_src = g["WqT"][:, 0:P].rearrange("(qc p) k -> p qc k", p=P)
    for qh in range(2):
        w = g["wq_pool"].tile([P, KC // 2, P], f32r, name="wq_sb")
        nc.scalar.dma_start(out=w, in_=wq_src[:, qh * 4 : (qh + 1) * 4, :])
        tiles.append(w)
    return tiles


def _phase_a(nc, g, b, qt_sb, wq0):
    """QmT[k, t] = sum_q WqT[q,k] * QT[q,t] + bq  (float32r)."""
    qmt_sb = g["qmt_pool"].tile([P, KT, TQ], f32r, name="qmt_sb")
    for kt in range(KT):
        wq_src = g["WqT"][:, kt * P : (kt + 1) * P].rearrange("(qc p) k -> p qc k", p=P)
        pa = g["ps"].tile([P, 2, 512], f32, tag="ps", name="pa")
        for qh in range(2):
            if kt == 0 and wq0 is not None:
                wq_sb = wq0[qh]
            else:
                wq_sb = g["wq_pool"].tile([P, KC // 2, P], f32r, name="wq_sb")
                nc.scalar.dma_start(
                    out=wq_sb, in_=wq_src[:, qh * 4 : (qh + 1) * 4, :]
                )
            for q4 in range(KC // 2):
                qc = qh * 4 + q4
                for th in range(2):
                    nc.tensor.matmul(
                        pa[:, th, :],
                        wq_sb[:, q4, :],
                        qt_sb[:, qc, th * 512 : (th + 1) * 512],
                        start=(qc == 0),
                        stop=(qc == KC - 1),
                    )
        nc.vector.tensor_scalar_add(
            qmt_sb[:, kt, :],
            pa.rearrange("p a b -> p (a b)"),
            g["bq_sb"][:, kt : kt + 1],
        )
    g["qmt_sb"] = qmt_sb


def _transpose_pb(nc, g, pb_sb, col, pt_sb):
    """16 PE transposes of one P t-tile (bf16) into PT[l, t] columns."""
    tp = g["ps"].tile([P, LC, P], bf16, tag="ps", name="tp")
    for lc in range(LC):
        nc.tensor.transpose(
            tp[:, lc, :], pb_sb[:, lc * P : (lc + 1) * P], g["ident"][:]
        )
    nc.scalar.activation(
        pt_sb[:, :, col * P : (col + 1) * P],
        tp[:, :, :],
        mybir.ActivationFunctionType.Copy,
    )


def _phase_b(nc, g, b, th, kt_sb):
    """scores (float32r) + softmax per t-tile; PE-transpose P into PT (bf16)."""
    pt_sb = g["pt_pool"].tile([P, LC, 512], bf16, name="pt_sb")
    v_pre = []
    for lc in range(2):  # prefetch first V chunks for the C phase that follows
        v_sb = g["v_pool"].tile([P, DV], bf16, name="v_sb")
        nc.gpsimd.dma_start(out=v_sb, in_=g["Vbf"][b, lc * P : (lc + 1) * P, :])
        v_pre.append(v_sb)
    g["v_pre"] = v_pre
    pending = None
    for tt in range(th * 4, th * 4 + 4):
        pb_ps = g["ps"].tile([P, LB, 512], f32, tag="ps", name="pb_ps")
        for kc in range(KC):
            lhsT = g["qmt_sb"][:, kc, tt * P : (tt + 1) * P]
            for lb in range(LB):
                nc.tensor.matmul(
                    pb_ps[:, lb, :],
                    lhsT,
                    kt_sb[:, kc, lb * 512 : (lb + 1) * 512],
                    start=(kc == 0),
                    stop=(kc == KC - 1),
                )
        if pending is not None:
            _transpose_pb(nc, g, pending[0], pending[1], pt_sb)
        flat = pb_ps.rearrange("p a b -> p (a b)")
        neg_m = g["small"].tile([P, 1], f32, name="neg_m")
        nc.vector.reduce_max(neg_m, flat, axis=mybir.AxisListType.X, negate=True)
        e_sb = g["e_pool"].tile([P, L], f32, name="e_sb")
        s_sum = g["small"].tile([P, 1], f32, name="s_sum")
        nc.scalar.activation(
            e_sb[:],
            flat,
            mybir.ActivationFunctionType.Exp,
            bias=neg_m[:],
            accum_out=s_sum[:],
        )
        r_sb = g["small"].tile([P, 1], f32, name="r_sb")
        nc.vector.reciprocal(r_sb, s_sum)
        # critical path to phase C: normalized bf16 copy straight from ACT
        pb_sb = g["pb_pool"].tile([P, L], bf16, name="pb_sb")
        nc.scalar.activation(
            pb_sb[:], e_sb[:], mybir.ActivationFunctionType.Copy, scale=r_sb[:]
        )
        pending = (pb_sb, tt - th * 4)
        # attnW output (f32), off the critical path
        nc.vector.tensor_scalar_mul(e_sb[:], e_sb[:], r_sb[:])
        nc.sync.dma_start(out=g["attnW"][b, tt * P : (tt + 1) * P, :], in_=e_sb)
    _transpose_pb(nc, g, pending[0], pending[1], pt_sb)
    return pt_sb


def _phase_c(nc, g, b, th, pt_sb):
    """attnT[v, t-half] = sum_l V[l,v] * P[t,l]  (bf16, PT resident in SBUF)."""
    attnt_sb = g["attnt_pool"].tile([P, VT, 512], bf16, name="attnt_sb")
    pc0 = g["ps"].tile([P, 4, 512], f32, tag="ps", name="pc0")
    pc1 = g["ps"].tile([P, 4, 512], f32, tag="ps", name="pc1")
    pcs = [pc0, pc1]
    for lc in range(LC):
        if lc < 2:
            v_sb = g["v_pre"][lc]
        else:
            v_sb = g["v_pool"].tile([P, DV], bf16, name="v_sb")
            nc.gpsimd.dma_start(
                out=v_sb, in_=g["Vbf"][b, lc * P : (lc + 1) * P, :]
            )
        for vt in range(VT):
            nc.tensor.matmul(
                pcs[vt // 4][:, vt % 4, :],
                v_sb[:, vt * P : (vt + 1) * P],
                pt_sb[:, lc, :],
                start=(lc == 0),
                stop=(lc == LC - 1),
            )
    for vt in range(VT):
        nc.scalar.activation(
            attnt_sb[:, vt, :],
            pcs[vt // 4][:, vt % 4, :],
            mybir.ActivationFunctionType.Copy,
        )
    return attnt_sb


def _phase_d(nc, g, b, th, attnt_sb):
    """out[t-half, o] = sum_c comb[c,t] * WcT[c,o] + bc  (bf16)."""
    for tq in [th]:
        pd0 = g["ps"].tile([P, 4, 512], f32, tag="ps", name="pd0")
        pd1 = g["ps"].tile([P, 4, 512], f32, tag="ps", name="pd1")
        pds = [pd0, pd1]
        for cb in range(CC // 2):
            wc_sb = g["wc_pool"].tile([P, 2, DOUT], bf16, name="wc_sb")
            nc.scalar.dma_start(
                out=wc_sb,
                in_=g["WcT"][cb * 2 * P : (cb + 1) * 2 * P, :].rearrange(
                    "(c p) o -> p c o", p=P
                ),
            )
            qq = None
            if cb * 2 < KC:
                qq = g["qtbf_pool"].tile([P, 2, 512], bf16, name="qq")
                nc.scalar.dma_start(
                    out=qq,
                    in_=g["QTbf"][
                        b, cb * 2 * P : (cb + 1) * 2 * P, tq * 512 : (tq + 1) * 512
                    ].rearrange("(c p) t -> p c t", p=P),
                )
            for cc in range(2):
                c = cb * 2 + cc
                for i in range(4):
                    tt = tq * 4 + i
                    if c < KC:
                        lhsT = qq[:, cc, i * P : (i + 1) * P]
                    else:
                        lhsT = attnt_sb[:, c - KC, i * P : (i + 1) * P]
                    for oc in range(OC):
                        j = i * OC + oc
                        nc.tensor.matmul(
                            pds[j // 4][:, j % 4, :],
                            lhsT,
                            wc_sb[:, cc, oc * 512 : (oc + 1) * 512],
                            start=(c == 0),
                            stop=(c == CC - 1),
                        )
        for i in range(4):
            tt = tq * 4 + i
            for oc in range(OC):
                j = i * OC + oc
                o_sb = g["out_pool"].tile([P, 512], f32, name="o_sb")
                nc.vector.tensor_add(
                    o_sb[:],
                    pds[j // 4][:, j % 4, :],
                    g["bc_sb"][:, oc * 512 : (oc + 1) * 512],
                )
                nc.sync.dma_start(
                    out=g["out"][b, tt * P : (tt + 1) * P, oc * 512 : (oc + 1) * 512],
                    in_=o_sb,
                )


def _build():
    nc = bacc.Bacc(
        "TRN2",
        target_bir_lowering=False,
        debug=False,
        enable_asserts=True,
        num_devices=NCORES,
    )
    g = {}
    g["QT"] = nc.dram_tensor("QT", [BLOC, DQ, TQ], f32r, kind="ExternalInput").ap()
    g["KTd"] = nc.dram_tensor("KTd", [BLOC, DK, L], f32r, kind="ExternalInput").ap()
    g["Vbf"] = nc.dram_tensor("Vbf", [BLOC, L, DV], bf16, kind="ExternalInput").ap()
    g["QTbf"] = nc.dram_tensor("QTbf", [BLOC, DQ, TQ], bf16, kind="ExternalInput").ap()
    g["WqT"] = nc.dram_tensor("WqT", [DQ, DK], f32r, kind="ExternalInput").ap()
    g["bq"] = nc.dram_tensor("bq", [DK], f32, kind="ExternalInput").ap()
    g["WcT"] = nc.dram_tensor("WcT", [DQ + DV, DOUT], bf16, kind="ExternalInput").ap()
    g["bc"] = nc.dram_tensor("bc", [1, DOUT], bf16, kind="ExternalInput").ap()
    g["attnW"] = nc.dram_tensor(
        "attnW", [BLOC, TQ, L], f32, kind="ExternalOutput"
    ).ap()
    g["out"] = nc.dram_tensor(
        "out", [BLOC, TQ, DOUT], f32, kind="ExternalOutput"
    ).ap()

    with tile.TileContext(nc) as tc, ExitStack() as ctx:
        for name, bufs, space in [
            ("const", 1, "SBUF"),
            ("qt_pool", 1, "SBUF"),
            ("kt_pool", 1, "SBUF"),
            ("qmt_pool", 1, "SBUF"),
            ("attnt_pool", 1, "SBUF"),
            ("wq_pool", 3, "SBUF"),
            ("e_pool", 2, "SBUF"),
            ("pb_pool", 2, "SBUF"),
            ("pt_pool", 1, "SBUF"),
            ("v_pool", 2, "SBUF"),
            ("qtbf_pool", 2, "SBUF"),
            ("wc_pool", 2, "SBUF"),
            ("out_pool", 3, "SBUF"),
            ("small", 4, "SBUF"),
            ("ps", 2, "PSUM"),
            ("dram", 2, "DRAM"),
        ]:
            g[name] = ctx.enter_context(tc.tile_pool(name=name, bufs=bufs, space=space))

        bq_sb = g["const"].tile([P, KT], f32)
        nc.sync.dma_start(out=bq_sb, in_=g["bq"].rearrange("(kt p) -> p kt", p=P))
        bc_sb = g["const"].tile([P, DOUT], bf16, name="bc_sb")
        nc.sync.dma_start(out=bc_sb, in_=g["bc"].to_broadcast([P, DOUT]))
        ident = g["const"].tile([P, P], bf16)
        make_identity(nc, ident)
        g["bq_sb"] = bq_sb
        g["bc_sb"] = bc_sb
        g["ident"] = ident

        qt_cur = _load_qt(nc, g, 0)
        kt_cur = _load_kt_first(nc, g, 0)
        _load_kt_rest(nc, g, 0, kt_cur)
        wq0 = None
        for b in range(BLOC):
            _phase_a(nc, g, b, qt_cur, wq0)
            if b + 1 < BLOC:
                qt_cur = _load_qt(nc, g, b + 1)  # prefetch during B/C/D
            kt_next = None
            for th in range(2):
                pt_sb = _phase_b(nc, g, b, th, kt_cur)
                attnt = _phase_c(nc, g, b, th, pt_sb)
                if th == 0:
                    wq0 = _prefetch_wq(nc, g) if b + 1 < BLOC else None
                elif b + 1 < BLOC:
                    # kt(b) slot free after B1; chunks move in the D1/A window
                    kt_next = _load_kt_first(nc, g, b + 1)
                _phase_d(nc, g, b, th, attnt)
            if kt_next is not None:
                _load_kt_rest(nc, g, b + 1, kt_next)
                kt_cur = kt_next

    nc.compile()
    return nc


def _round_f32r(x):
    """Round float32 to the f32r grid (11 stored mantissa bits, RNE)."""
    b = np.ascontiguousarray(x, np.float32).view(np.uint32).astype(np.uint64)
    r = (b + 0x7FF + ((b >> 12) & 1)) & ~np.uint64(0xFFF)
    return r.astype(np.uint32).view(np.float32).reshape(x.shape)


def kernel(Q, K, V, Wq, bq, Wc, bc):
    global _LAST_IN_MAPS
    Q = np.asarray(Q, np.float32)
    K = np.asarray(K, np.float32)
    V = np.asarray(V, np.float32)
    Wq = np.asarray(Wq, np.float32)
    bq = np.asarray(bq, np.float32)
    Wc = np.asarray(Wc, np.float32)
    bc = np.asarray(bc, np.float32)

    if "nc" not in _NC_CACHE:
        _NC_CACHE["nc"] = _build()
    nc = _NC_CACHE["nc"]

    QT = _round_f32r(np.ascontiguousarray(Q.transpose(0, 2, 1)))
    KT_h = _round_f32r(np.ascontiguousarray(K.transpose(0, 2, 1)))
    Vbf = V.astype(ml_dtypes.bfloat16)
    QTbf = np.ascontiguousarray(Q.transpose(0, 2, 1)).astype(ml_dtypes.bfloat16)
    WqT = _round_f32r(np.ascontiguousarray(Wq.T))
    WcT = np.ascontiguousarray(Wc.T).astype(ml_dtypes.bfloat16)
    bc2 = bc.reshape(1, DOUT)

    in_maps = []
    for c in range(NCORES):
        s = slice(c * BLOC, (c + 1) * BLOC)
        in_maps.append(
            {
                "QT": QT[s],
                "KTd": KT_h[s],
                "Vbf": Vbf[s],
                "QTbf": QTbf[s],
                "WqT": WqT,
                "bq": bq,
                "WcT": WcT,
                "bc": bc2.astype(ml_dtypes.bfloat16),
            }
        )

    _LAST_IN_MAPS = in_maps
    res = bass_utils.run_bass_kernel_spmd(nc, in_maps, core_ids=list(range(NCORES)))
    out = np.concatenate([res.results[c]["out"] for c in range(NCORES)], axis=0)
    attn = np.concatenate([res.results[c]["attnW"] for c in range(NCORES)], axis=0)
    return out, attn
